# revision 34
# baseline (speedup 1.0000x reference)
"""Trainium2 Bass kernel for BertMultiHeadAttention (B=4, S=1024, D=768, H=12).

Sharding: 8 cores = 4 batches x 2 head-groups (6 heads each).
Phase order per core: QKV -> chainB (transposed scores -> exp -> ctx with
fused rowsums via a ones-column in V) -> normalize ctx -> output projection
partial -> ReduceScatter over core pairs -> chainA (scores -> exp -> softmax
probs output; overlaps the collective) -> residual+LayerNorm on this core's
half of the tokens.

Returns (normed [4,1024,768] f32, attn_probs [4,12,1024,1024] f32).
"""

import math
import os

import ml_dtypes
import numpy as np

import concourse.bass as bass
import concourse.mybir as mybir
import concourse.tile as tile
from concourse import bacc
from concourse.bass_utils import run_bass_kernel_spmd
from concourse.masks import make_identity

f32 = mybir.dt.float32
f32r = mybir.dt.float32r
f16 = mybir.dt.float16
bf16 = mybir.dt.bfloat16
AF = mybir.ActivationFunctionType
ALU = mybir.AluOpType

B, S, D, H = 4, 1024, 768, 12
HD = D // H          # 64
HG = H // 2          # 6 heads per group
DG = HG * HD         # 384
SH = S // 2          # 512 tokens per core for LN
NB = S // 128        # 8 q/k blocks
MASKVAL = -100000.0
SCALE = float(1.0 / math.sqrt(float(D)))
EPS = 1e-12

_CACHE = {}
_LAST_RES = None


def _build(flags):
    """Build the SPMD Bass program. flags = (qb_nz, kb_nz, vb_nz, mask_nz, ln_nz)."""
    qb_nz, kb_nz, vb_nz, mask_nz, ln_nz = flags
    nc = bacc.Bacc("TRN2", target_bir_lowering=False, debug=False, num_devices=8)

    xT_d = nc.dram_tensor("xT", [D, S], f16, kind="ExternalInput")
    wqT_d = nc.dram_tensor("wqT", [D, DG], f16, kind="ExternalInput")
    wkT_d = nc.dram_tensor("wkT", [D, DG], f16, kind="ExternalInput")
    wvT_d = nc.dram_tensor("wvT", [D, DG], f16, kind="ExternalInput")
    woT_d = nc.dram_tensor("woT", [DG, D], f16, kind="ExternalInput")
    emb_d = nc.dram_tensor("emb_resid", [SH, D], f32, kind="ExternalInput")
    if qb_nz:
        qb_d = nc.dram_tensor("qb", [DG, 1], f32, kind="ExternalInput")
    if kb_nz:
        kb_d = nc.dram_tensor("kb", [DG, 1], f32, kind="ExternalInput")
    if vb_nz:
        vb_d = nc.dram_tensor("vb", [1, DG], f32, kind="ExternalInput")
    if mask_nz:
        mb_row_d = nc.dram_tensor("mb_row", [1, S], f32, kind="ExternalInput")
        mb_colT_d = nc.dram_tensor("mb_colT", [128, NB], f32, kind="ExternalInput")
    if ln_nz:
        lnw_d = nc.dram_tensor("lnw", [2, D], f32, kind="ExternalInput")

    probs_d = nc.dram_tensor("probs", [HG, S, S], f32, kind="ExternalOutput")
    normed_d = nc.dram_tensor("normed", [SH, D], f32, kind="ExternalOutput")
    _dbg = os.environ.get("MHA_DEBUG", "0") == "1"
    if _dbg:
        dbg_ctxT_d = nc.dram_tensor("dbg_ctxT", [6, 64, S], f32, kind="ExternalOutput")
        dbg_partial_d = nc.dram_tensor("dbg_partial", [S, D], f32, kind="ExternalOutput")
        dbg_ar_d = nc.dram_tensor("dbg_ar", [SH, D], f32, kind="ExternalOutput")

    with tile.TileContext(nc) as tc:
        with (
            tc.tile_pool(name="sb", bufs=1) as sb,
            tc.tile_pool(name="rot", bufs=1) as rot,
            tc.tile_pool(name="ps", bufs=1, space="PSUM") as ps,
            tc.tile_pool(name="dram", bufs=1, space="DRAM") as dram,
        ):
            # ---------- constants ----------
            ident_bf = sb.tile([128, 128], bf16, tag="ident_bf")
            make_identity(nc, ident_bf[:])
            # cmask[q,k]: 0 where k<=q else MASKVAL (chain A diagonal blocks)
            cmask = sb.tile([128, 128], bf16, tag="cmask")
            nc.gpsimd.memset(cmask[:], 0.0)
            nc.gpsimd.affine_select(
                out=cmask[:], in_=cmask[:], compare_op=ALU.is_ge,
                fill=MASKVAL, base=0, pattern=[[-1, 128]], channel_multiplier=1,
            )

            # ---------- load inputs ----------
            xT = [sb.tile([128, S], f16, tag=f"xT{e}", name=f"xT{e}") for e in range(6)]
            for e in range(6):
                nc.sync.dma_start(xT[e][:], xT_d.ap()[e * 128:(e + 1) * 128, :])
            wqT = [sb.tile([128, DG], f16, tag=f"wqT{e}", name=f"wqT{e}") for e in range(6)]
            wkT = [sb.tile([128, DG], f16, tag=f"wkT{e}", name=f"wkT{e}") for e in range(6)]
            wvT = [sb.tile([128, DG], f16, tag=f"wvT{e}", name=f"wvT{e}") for e in range(6)]
            for w_sb, w_d in ((wqT, wqT_d), (wkT, wkT_d), (wvT, wvT_d)):
                for e in range(6):
                    nc.sync.dma_start(w_sb[e][:], w_d.ap()[e * 128:(e + 1) * 128, :])
            woT = [sb.tile([64, D], f16, tag=f"woT{h}", name=f"woT{h}") for h in range(6)]
            for h in range(6):
                nc.sync.dma_start(woT[h][:], woT_d.ap()[h * 64:(h + 1) * 64, :])
            xTh = xT

            if qb_nz:
                qb_sb = sb.tile([128, 3], f32, tag="qb_sb")
                nc.sync.dma_start(qb_sb[:], qb_d.ap().rearrange("(a b) c -> b (a c)", a=3))
            if kb_nz:
                kb_sb = sb.tile([128, 3], f32, tag="kb_sb")
                nc.sync.dma_start(kb_sb[:], kb_d.ap().rearrange("(a b) c -> b (a c)", a=3))
            if vb_nz:
                vb_sb = sb.tile([128, DG], f32, tag="vb_sb")
                a = vb_d.ap()
                nc.sync.dma_start(vb_sb[:], bass.AP(a.tensor, a.offset, [[0, 128], [1, DG]]))
            if mask_nz:
                mb_row = sb.tile([1, S], f32r, tag="mb_row")
                nc.sync.dma_start(mb_row[:], mb_row_d.ap().bitcast(f32r))
                mb_colT = sb.tile([128, NB], f32, tag="mb_colT")
                nc.sync.dma_start(mb_colT[:], mb_colT_d.ap())
                ones1 = sb.tile([1, 128], f32r, tag="ones1")
                nc.vector.memset(ones1[:], 1.0)

            # ---------- phase 1: QKV ----------
            QTh = [sb.tile([128, S], f16, tag=f"QTh{d}", name=f"QTh{d}") for d in range(3)]
            KTh = [sb.tile([128, S], f16, tag=f"KTh{d}", name=f"KTh{d}") for d in range(3)]
            # V_ext: per k-tile [128, 6*65]; head h at cols h*65..h*65+63, col h*65+64 = 1.0
            V = [sb.tile([128, 6 * 65], f16, tag=f"V{t}", name=f"V{t}") for t in range(NB)]
            for t in range(NB):
                nc.vector.memset(
                    V[t][:].rearrange("p (h c) -> p h c", c=65)[:, :, 64:65]
                    .bitcast(mybir.dt.uint16), 0x3C00)

            for (w_sb, out_h, bias_nz, bias_tile) in (
                (wqT, QTh, qb_nz, qb_sb if qb_nz else None),
                (wkT, KTh, kb_nz, kb_sb if kb_nz else None),
            ):
                for d in range(3):
                    for nch in range(2):
                        pq = ps.tile([128, 512], f32, tag="pu", bufs=8)
                        for e in range(6):
                            nc.tensor.matmul(
                                pq[:], w_sb[e][:, d * 128:(d + 1) * 128],
                                xT[e][:, nch * 512:(nch + 1) * 512],
                                start=(e == 0), stop=(e == 5),
                            )
                        dst = out_h[d][:, nch * 512:(nch + 1) * 512]
                        if bias_nz:
                            nc.scalar.activation(dst, pq[:], AF.Identity, bias=bias_tile[:, d:d + 1])
                        else:
                            nc.vector.tensor_copy(dst, pq[:])
            for t in range(NB):
                pv = ps.tile([128, DG], f32, tag="pu", bufs=8)
                for e in range(6):
                    nc.tensor.matmul(
                        pv[:], xTh[e][:, t * 128:(t + 1) * 128], wvT[e][:],
                        start=(e == 0), stop=(e == 5),
                    )
                vdst = V[t][:].rearrange("p (h c) -> p h c", c=65)[:, :, 0:64]
                vsrc = pv[:].rearrange("p (h c) -> p h c", c=64)
                if vb_nz:
                    nc.vector.tensor_tensor(
                        vdst, vsrc, vb_sb[:].rearrange("p (h c) -> p h c", c=64), op=ALU.add)
                else:
                    nc.vector.tensor_copy(vdst, vsrc)

            # ---------- warm-burst: dense MMs to re-arm the PE clock ----------
            wsrc = sb.tile([128, 512], bf16, tag="wsrc")
            nc.vector.memset(wsrc[:].bitcast(mybir.dt.uint16), 0x3C00)

            def warm_burst(nmm=8):
                pw = ps.tile([128, 512], f32, tag="pu", bufs=8)
                for r in range(nmm):
                    nc.tensor.matmul(pw[:], wsrc[:, 0:128], wsrc[:],
                                     start=(r == 0), stop=(r == nmm - 1))

            # ---------- phase 2: chainB, heads interleaved in pairs ----------
            ctxU = [sb.tile([65, S], f16, tag=f"ctxU{h}", name=f"ctxU{h}") for h in range(6)]
            ctxT = [u[0:64, :] for u in ctxU]
            rr_dram = dram.tile([6, S], f16)
            rr2_dram = dram.tile([6, S], f16)

            def chainB_pair(h0):
                pair = (h0, h0 + 1)
                pCc = {}
                for h in pair:
                    pCc[(h, 0)] = ps.tile([65, 512], f32, tag="pu", bufs=8, name=f"pC{h}c0")
                    pCc[(h, 1)] = ps.tile([65, 512], f32, tag="pu", bufs=8, name=f"pC{h}c1")
                seen = {(h, c): 0 for h in pair for c in (0, 1)}
                total = {0: 4, 1: 8}
                for j in range(NB):
                    for h in pair:
                        dt_, hb = h // 2, (h % 2) * 64
                        qh = QTh[dt_][hb:hb + 64, :]
                        kh = KTh[dt_][hb:hb + 64, :]
                        q0 = j * 128
                        first_slab = True
                        while q0 < S:
                            ch = q0 // 512
                            hi = (ch + 1) * 512
                            n = hi - q0
                            pST = ps.tile([128, 512], f32, tag="pu", bufs=8)
                            nc.tensor.matmul(
                                pST[:, 0:n], kh[:, j * 128:(j + 1) * 128], qh[:, q0:q0 + n],
                                start=True, stop=True,
                            )
                            ET = rot.tile([128, 512], f16, tag="ET", bufs=3)
                            if mask_nz:
                                nc.scalar.activation(
                                    ET[:, 0:n], pST[:, 0:n], AF.Exp, scale=SCALE,
                                    bias=mb_colT[:, j:j + 1],
                                )
                            else:
                                nc.scalar.activation(ET[:, 0:n], pST[:, 0:n], AF.Exp, scale=SCALE)
                            if first_slab:
                                nc.gpsimd.affine_select(
                                    out=ET[:, 0:128], in_=ET[:, 0:128],
                                    compare_op=ALU.is_ge, fill=0.0,
                                    base=0, pattern=[[1, 128]], channel_multiplier=-1,
                                )
                            seen[(h, ch)] += 1
                            nc.tensor.matmul(
                                pCc[(h, ch)][:, q0 - ch * 512:q0 - ch * 512 + n],
                                V[j][:, h * 65:(h + 1) * 65],
                                ET[:, 0:n],
                                start=(j == 0),
                                stop=(seen[(h, ch)] == total[ch]),
                            )
                            first_slab = False
                            q0 = hi
                for h in pair:
                    for c in (0, 1):
                        nc.vector.tensor_copy(
                            ctxU[h][:, c * 512:(c + 1) * 512], pCc[(h, c)][:])
                    nc.sync.dma_start(rr_dram[h:h + 1, :], ctxU[h][64:65, :])

            for p in range(3):
                chainB_pair(2 * p)

            # ---------- chainA machinery ----------
            rsums = sb.tile([128, 48], f32, tag="rsums")
            recip = sb.tile([128, 48], f32, tag="recip")

            def chainA_pair(h0):
                for i in range(NB):
                    w = (i + 1) * 128
                    for h in (h0, h0 + 1):
                        dt_, hb = h // 2, (h % 2) * 64
                        qh = QTh[dt_][hb:hb + 64, :]
                        kh = KTh[dt_][hb:hb + 64, :]
                        dlo = i * 128
                        col = h * 8 + i
                        E = rot.tile([128, 1024], f32, tag="E", bufs=3, name=f"E{h}_{i}")
                        nparts = 0
                        for k0 in range(0, w, 512):
                            n = min(512, w - k0)
                            diag_here = dlo >= k0 and dlo < k0 + n
                            pS = ps.tile([128, 512], f32, tag="pu", bufs=8)
                            nc.tensor.matmul(
                                pS[:, 0:n], qh[:, i * 128:(i + 1) * 128], kh[:, k0:k0 + n],
                                start=True, stop=not (diag_here or mask_nz),
                            )
                            if mask_nz:
                                nc.tensor.matmul(
                                    pS[:, 0:n], ones1[:, 0:128], mb_row[:, k0:k0 + n],
                                    start=False, stop=not diag_here,
                                )
                            if diag_here:
                                nc.tensor.matmul(
                                    pS[:, dlo - k0:dlo - k0 + 128], ident_bf[:], cmask[:],
                                    start=False, stop=True,
                                )
                            acc = rsums[:, col:col + 1] if k0 == 0 else rspart[:, h % 2:h % 2 + 1]
                            if k0 > 0:
                                nparts += 1
                            nc.scalar.activation(
                                E[:, k0:k0 + n], pS[:, 0:n], AF.Exp, scale=SCALE,
                                accum_out=acc,
                            )
                        if nparts:
                            nc.vector.tensor_tensor(
                                rsums[:, col:col + 1], rsums[:, col:col + 1],
                                rspart[:, h % 2:h % 2 + 1], op=ALU.add)
                        nc.vector.reciprocal(recip[:, col:col + 1], rsums[:, col:col + 1])
                        P = rot.tile([128, 1024], f32, tag="P", bufs=4, name=f"P{h}_{i}")
                        nc.vector.tensor_scalar(
                            P[:, 0:w], E[:, 0:w], recip[:, col:col + 1], None, op0=ALU.mult,
                        )
                        nc.sync.dma_start(
                            probs_d.ap()[h, i * 128:(i + 1) * 128, 0:w], P[:, 0:w])

            rspart = sb.tile([128, 2], f32, tag="rspart")
            for p in range(3):
                chainA_pair(2 * p)
            warm_burst()

            # ---------- normalize ctx ----------
            rsum6h = sb.tile([6, S], f16, tag="rsum6h")
            rsum6 = sb.tile([6, S], f32, tag="rsum6")
            rrec6 = sb.tile([6, S], f32, tag="rrec6")
            rscr6 = sb.tile([6, S], f32, tag="rscr6")
            rrec6h = sb.tile([6, S], f16, tag="rrec6h")
            nc.sync.dma_start(rsum6h[:], rr_dram[:])
            nc.vector.tensor_copy(rsum6[:], rsum6h[:])
            nc.vector.reciprocal_approx_accurate(rrec6[:], rsum6[:], rscr6[:])
            nc.vector.tensor_copy(rrec6h[:], rrec6[:])
            nc.sync.dma_start(rr2_dram[:], rrec6h[:])
            ra = rr2_dram[:]
            for h in range(6):
                rb = rot.tile([64, S], f16, tag="rb", bufs=2)
                nc.sync.dma_start(
                    rb[:], bass.AP(ra.tensor, ra.offset + h * S, [[0, 64], [1, S]]))
                nc.vector.tensor_tensor(ctxT[h], ctxU[h][0:64, :], rb[:], op=ALU.mult)
                if _dbg:
                    dbgc = rot.tile([64, S], f32, tag="dbgc", bufs=1)
                    nc.vector.tensor_copy(dbgc[:], ctxT[h])
                    nc.sync.dma_start(dbg_ctxT_d.ap()[h], dbgc[:])

            # ---------- projection + ReduceScatter ----------
            partial_dram = dram.tile([S, D], f16)
            ar_dram = dram.tile([SH, D], f16)
            for t in range(NB):
                part = rot.tile([128, D], f16, tag="part", bufs=2)
                for e0, e1 in ((0, 512), (512, 768)):
                    pP = ps.tile([128, 512], f32, tag="pu", bufs=8)
                    for h in range(6):
                        nc.tensor.matmul(
                            pP[:, 0:e1 - e0],
                            ctxT[h][:, t * 128:(t + 1) * 128],
                            woT[h][:, e0:e1],
                            start=(h == 0), stop=(h == 5),
                        )
                    nc.vector.tensor_copy(part[:, e0:e1], pP[:, 0:e1 - e0])
                nc.sync.dma_start(partial_dram[t * 128:(t + 1) * 128, :], part[:])
                if _dbg:
                    nc.sync.dma_start(dbg_partial_d.ap()[t * 128:(t + 1) * 128, :], part[:])

            nc.gpsimd.collective_compute(
                "ReduceScatter", ALU.add,
                replica_groups=[[0, 1], [2, 3], [4, 5], [6, 7]],
                ins=[partial_dram.opt()],
                outs=[ar_dram.opt()],
            )

            # ---------- LayerNorm ----------
            if ln_nz:
                lng = sb.tile([128, D], f32, tag="lng")
                lnb = sb.tile([128, D], f32, tag="lnb")
                a = lnw_d.ap()
                nc.sync.dma_start(lng[:], bass.AP(a.tensor, a.offset, [[0, 128], [1, D]]))
                nc.sync.dma_start(lnb[:], bass.AP(a.tensor, a.offset + D, [[0, 128], [1, D]]))

            for t in range(4):
                ar0 = rot.tile([128, D], f16, tag="ar0", bufs=4)
                nc.sync.dma_start(ar0[:], ar_dram[t * 128:(t + 1) * 128, :])
                emb_sb = rot.tile([128, D], f32, tag="emb_sb", bufs=2)
                nc.sync.dma_start(emb_sb[:], emb_d.ap()[t * 128:(t + 1) * 128, :])
                x_sb = rot.tile([128, D], f32, tag="x_sb", bufs=2)
                sum_x = rot.tile([128, 1], f32, tag="sum_x", bufs=2)
                nc.vector.scalar_tensor_tensor(
                    x_sb[:], ar0[:], 1.0, emb_sb[:], op0=ALU.mult, op1=ALU.add,
                    accum_out=sum_x[:],
                )
                if _dbg:
                    nc.vector.tensor_tensor(ar0[:], x_sb[:], emb_sb[:], op=ALU.subtract)
                    nc.sync.dma_start(dbg_ar_d.ap()[t * 128:(t + 1) * 128, :], ar0[:])
                mu_neg = rot.tile([128, 1], f32, tag="mu_neg", bufs=4)
                nc.vector.tensor_scalar(mu_neg[:], sum_x[:], -1.0 / D, None, op0=ALU.mult)
                sq = rot.tile([128, D], f16, tag="part", bufs=2)
                svar = rot.tile([128, 1], f32, tag="svar", bufs=2)
                nc.scalar.activation(
                    sq[:], x_sb[:], AF.Square, bias=mu_neg[:], accum_out=svar[:],
                )
                nc.vector.tensor_scalar(svar[:], svar[:], 1.0 / D, EPS, op0=ALU.mult, op1=ALU.add)
                srstd = rot.tile([128, 1], f32, tag="srstd", bufs=2)
                nc.scalar.activation(srstd[:], svar[:], AF.Sqrt)
                nc.vector.reciprocal(srstd[:], srstd[:])
                nrm = rot.tile([128, D], f32, tag="nrm", bufs=2)
                nc.vector.tensor_scalar(
                    nrm[:], x_sb[:], mu_neg[:], srstd[:],
                    op0=ALU.add, op1=ALU.mult,
                )
                if ln_nz:
                    nc.vector.tensor_tensor(nrm[:], nrm[:], lng[:], op=ALU.mult)
                    nc.vector.tensor_tensor(nrm[:], nrm[:], lnb[:], op=ALU.add)
                nc.sync.dma_start(normed_d.ap()[t * 128:(t + 1) * 128, :], nrm[:])

    nc.compile()
    return nc


def kernel(embeddings, padding_mask, wq_w, wq_b, wk_w, wk_b, wv_w, wv_b,
           wo_w, wo_b, ln_g, ln_b):
    global _LAST_RES
    embeddings = np.asarray(embeddings, dtype=np.float32)
    padding_mask = np.asarray(padding_mask, dtype=np.float32)
    wq_w = np.asarray(wq_w, dtype=np.float32)
    wk_w = np.asarray(wk_w, dtype=np.float32)
    wv_w = np.asarray(wv_w, dtype=np.float32)
    wo_w = np.asarray(wo_w, dtype=np.float32)
    wq_b = np.asarray(wq_b, dtype=np.float32)
    wk_b = np.asarray(wk_b, dtype=np.float32)
    wv_b = np.asarray(wv_b, dtype=np.float32)
    wo_b = np.asarray(wo_b, dtype=np.float32)
    ln_g = np.asarray(ln_g, dtype=np.float32)
    ln_b = np.asarray(ln_b, dtype=np.float32)

    flags = (
        bool(np.any(wq_b != 0)), bool(np.any(wk_b != 0)), bool(np.any(wv_b != 0)),
        bool(np.any(padding_mask != 1.0)),
        bool(np.any(ln_g != 1.0) or np.any(ln_b != 0.0)),
    )
    if flags not in _CACHE:
        _CACHE[flags] = _build(flags)
    nc = _CACHE[flags]
    qb_nz, kb_nz, vb_nz, mask_nz, ln_nz = flags

    in_maps = []
    for c in range(8):
        b, g = c // 2, c % 2
        hsel = slice(g * HG * HD, (g + 1) * HG * HD)
        m = {
            "xT": np.ascontiguousarray(embeddings[b].T.astype(np.float16)),
            "wqT": np.ascontiguousarray(wq_w[hsel, :].T.astype(np.float16)),
            "wkT": np.ascontiguousarray(wk_w[hsel, :].T.astype(np.float16)),
            "wvT": np.ascontiguousarray(wv_w[hsel, :].T.astype(np.float16)),
            "woT": np.ascontiguousarray(wo_w[:, hsel].T.astype(np.float16)),
            "emb_resid": np.ascontiguousarray(
                embeddings[b, g * SH:(g + 1) * SH, :] + wo_b[None, :]),
        }
        if qb_nz:
            m["qb"] = np.ascontiguousarray(wq_b[hsel].reshape(DG, 1))
        if kb_nz:
            m["kb"] = np.ascontiguousarray(wk_b[hsel].reshape(DG, 1))
        if vb_nz:
            m["vb"] = np.ascontiguousarray(wv_b[hsel].reshape(1, DG))
        if mask_nz:
            mb = (1.0 - padding_mask[b]) * (-10000.0) / SCALE
            m["mb_row"] = np.ascontiguousarray(mb.reshape(1, S))
            m["mb_colT"] = np.ascontiguousarray(
                ((1.0 - padding_mask[b]) * (-10000.0)).reshape(NB, 128).T)
        if ln_nz:
            m["lnw"] = np.ascontiguousarray(np.stack([ln_g, ln_b]))
        in_maps.append(m)

    res = run_bass_kernel_spmd(nc, in_maps, core_ids=list(range(8)))
    _LAST_RES = res

    normed = np.zeros((B, S, D), dtype=np.float32)
    attn_probs = np.zeros((B, H, S, S), dtype=np.float32)
    for c in range(8):
        b, g = c // 2, c % 2
        attn_probs[b, g * HG:(g + 1) * HG] = res.results[c]["probs"]
        normed[b, g * SH:(g + 1) * SH] = res.results[c]["normed"]
    return normed, attn_probs


# revision 35
# speedup vs baseline: 1.1605x; 1.1605x over previous
"""Trainium2 Bass kernel for BertMultiHeadAttention (B=4, S=1024, D=768, H=12).

Sharding: 8 cores = 4 batches x 2 head-groups (6 heads each).
Phase order per core: QKV -> chainB (transposed scores -> exp -> ctx with
fused rowsums via a ones-column in V) -> normalize ctx -> output projection
partial -> ReduceScatter over core pairs -> chainA (scores -> exp -> softmax
probs output; overlaps the collective) -> residual+LayerNorm on this core's
half of the tokens.

Returns (normed [4,1024,768] f32, attn_probs [4,12,1024,1024] f32).
"""

import math
import os

import ml_dtypes
import numpy as np

import concourse.bass as bass
import concourse.mybir as mybir
import concourse.tile as tile
from concourse import bacc
from concourse.bass_utils import run_bass_kernel_spmd
from concourse.masks import make_identity

f32 = mybir.dt.float32
f32r = mybir.dt.float32r
f16 = mybir.dt.float16
bf16 = mybir.dt.bfloat16
AF = mybir.ActivationFunctionType
ALU = mybir.AluOpType

B, S, D, H = 4, 1024, 768, 12
HD = D // H          # 64
HG = H // 2          # 6 heads per group
DG = HG * HD         # 384
SH = S // 2          # 512 tokens per core for LN
NB = S // 128        # 8 q/k blocks
MASKVAL = -100000.0
SCALE = float(1.0 / math.sqrt(float(D)))
EPS = 1e-12

_CACHE = {}
_LAST_RES = None


def _build(flags):
    """Build the SPMD Bass program. flags = (qb_nz, kb_nz, vb_nz, mask_nz, ln_nz)."""
    qb_nz, kb_nz, vb_nz, mask_nz, ln_nz = flags
    nc = bacc.Bacc("TRN2", target_bir_lowering=False, debug=False, num_devices=8)

    xT_d = nc.dram_tensor("xT", [D, S], f16, kind="ExternalInput")
    wqT_d = nc.dram_tensor("wqT", [D, DG], f16, kind="ExternalInput")
    wkT_d = nc.dram_tensor("wkT", [D, DG], f16, kind="ExternalInput")
    wvT_d = nc.dram_tensor("wvT", [D, DG], f16, kind="ExternalInput")
    woT_d = nc.dram_tensor("woT", [DG, D], f16, kind="ExternalInput")
    emb_d = nc.dram_tensor("emb_resid", [SH, D], f32, kind="ExternalInput")
    if qb_nz:
        qb_d = nc.dram_tensor("qb", [DG, 1], f32, kind="ExternalInput")
    if kb_nz:
        kb_d = nc.dram_tensor("kb", [DG, 1], f32, kind="ExternalInput")
    if vb_nz:
        vb_d = nc.dram_tensor("vb", [1, DG], f32, kind="ExternalInput")
    if mask_nz:
        mb_row_d = nc.dram_tensor("mb_row", [1, S], f32, kind="ExternalInput")
        mb_colT_d = nc.dram_tensor("mb_colT", [128, NB], f32, kind="ExternalInput")
    if ln_nz:
        lnw_d = nc.dram_tensor("lnw", [2, D], f32, kind="ExternalInput")

    probs_d = nc.dram_tensor("probs", [HG, S, S], f32, kind="ExternalOutput")
    normed_d = nc.dram_tensor("normed", [SH, D], f32, kind="ExternalOutput")
    _dbg = os.environ.get("MHA_DEBUG", "0") == "1"
    if _dbg:
        dbg_ctxT_d = nc.dram_tensor("dbg_ctxT", [6, 64, S], f32, kind="ExternalOutput")
        dbg_partial_d = nc.dram_tensor("dbg_partial", [S, D], f32, kind="ExternalOutput")
        dbg_ar_d = nc.dram_tensor("dbg_ar", [SH, D], f32, kind="ExternalOutput")

    with tile.TileContext(nc) as tc:
        with (
            tc.tile_pool(name="sb", bufs=1) as sb,
            tc.tile_pool(name="rot", bufs=1) as rot,
            tc.tile_pool(name="ps", bufs=1, space="PSUM") as ps,
            tc.tile_pool(name="dram", bufs=1, space="DRAM") as dram,
        ):
            # ---------- constants ----------
            ident_bf = sb.tile([128, 128], bf16, tag="ident_bf")
            make_identity(nc, ident_bf[:])
            # cmask[q,k]: 0 where k<=q else MASKVAL (chain A diagonal blocks)
            cmask = sb.tile([128, 128], bf16, tag="cmask")
            nc.gpsimd.memset(cmask[:], 0.0)
            nc.gpsimd.affine_select(
                out=cmask[:], in_=cmask[:], compare_op=ALU.is_ge,
                fill=MASKVAL, base=0, pattern=[[-1, 128]], channel_multiplier=1,
            )

            # ---------- load inputs ----------
            xT = [sb.tile([128, S], f16, tag=f"xT{e}", name=f"xT{e}") for e in range(6)]
            for e in range(6):
                nc.sync.dma_start(xT[e][:], xT_d.ap()[e * 128:(e + 1) * 128, :])
            wqT = [sb.tile([128, DG], f16, tag=f"wqT{e}", name=f"wqT{e}") for e in range(6)]
            wkT = [sb.tile([128, DG], f16, tag=f"wkT{e}", name=f"wkT{e}") for e in range(6)]
            wvT = [sb.tile([128, DG], f16, tag=f"wvT{e}", name=f"wvT{e}") for e in range(6)]
            for w_sb, w_d in ((wqT, wqT_d), (wkT, wkT_d), (wvT, wvT_d)):
                for e in range(6):
                    nc.sync.dma_start(w_sb[e][:], w_d.ap()[e * 128:(e + 1) * 128, :])
            woT = [sb.tile([64, D], f16, tag=f"woT{h}", name=f"woT{h}") for h in range(6)]
            for h in range(6):
                nc.sync.dma_start(woT[h][:], woT_d.ap()[h * 64:(h + 1) * 64, :])
            xTh = xT

            if qb_nz:
                qb_sb = sb.tile([128, 3], f32, tag="qb_sb")
                nc.sync.dma_start(qb_sb[:], qb_d.ap().rearrange("(a b) c -> b (a c)", a=3))
            if kb_nz:
                kb_sb = sb.tile([128, 3], f32, tag="kb_sb")
                nc.sync.dma_start(kb_sb[:], kb_d.ap().rearrange("(a b) c -> b (a c)", a=3))
            if vb_nz:
                vb_sb = sb.tile([128, DG], f32, tag="vb_sb")
                a = vb_d.ap()
                nc.sync.dma_start(vb_sb[:], bass.AP(a.tensor, a.offset, [[0, 128], [1, DG]]))
            if mask_nz:
                mb_row = sb.tile([1, S], f32r, tag="mb_row")
                nc.sync.dma_start(mb_row[:], mb_row_d.ap().bitcast(f32r))
                mb_colT = sb.tile([128, NB], f32, tag="mb_colT")
                nc.sync.dma_start(mb_colT[:], mb_colT_d.ap())
                ones1 = sb.tile([1, 128], f32r, tag="ones1")
                nc.vector.memset(ones1[:], 1.0)

            # ---------- phase 1: QKV ----------
            QTh = [sb.tile([128, S], f16, tag=f"QTh{d}", name=f"QTh{d}") for d in range(3)]
            KTh = [sb.tile([128, S], f16, tag=f"KTh{d}", name=f"KTh{d}") for d in range(3)]
            # V_ext: per k-tile [128, 6*65]; head h at cols h*65..h*65+63, col h*65+64 = 1.0
            V = [sb.tile([128, 6 * 65], f16, tag=f"V{t}", name=f"V{t}") for t in range(NB)]
            for t in range(NB):
                nc.vector.memset(
                    V[t][:].rearrange("p (h c) -> p h c", c=65)[:, :, 64:65]
                    .bitcast(mybir.dt.uint16), 0x3C00)

            for (w_sb, out_h, bias_nz, bias_tile) in (
                (wqT, QTh, qb_nz, qb_sb if qb_nz else None),
                (wkT, KTh, kb_nz, kb_sb if kb_nz else None),
            ):
                for d in range(3):
                    for nch in range(2):
                        pq = ps.tile([128, 512], f32, tag="pu", bufs=8)
                        for e in range(6):
                            nc.tensor.matmul(
                                pq[:], w_sb[e][:, d * 128:(d + 1) * 128],
                                xT[e][:, nch * 512:(nch + 1) * 512],
                                start=(e == 0), stop=(e == 5),
                            )
                        dst = out_h[d][:, nch * 512:(nch + 1) * 512]
                        if bias_nz:
                            nc.scalar.activation(dst, pq[:], AF.Identity, bias=bias_tile[:, d:d + 1])
                        else:
                            nc.vector.tensor_copy(dst, pq[:])
            for t in range(NB):
                pv = ps.tile([128, DG], f32, tag="pu", bufs=8)
                for e in range(6):
                    nc.tensor.matmul(
                        pv[:], xTh[e][:, t * 128:(t + 1) * 128], wvT[e][:],
                        start=(e == 0), stop=(e == 5),
                    )
                vdst = V[t][:].rearrange("p (h c) -> p h c", c=65)[:, :, 0:64]
                vsrc = pv[:].rearrange("p (h c) -> p h c", c=64)
                if vb_nz:
                    nc.vector.tensor_tensor(
                        vdst, vsrc, vb_sb[:].rearrange("p (h c) -> p h c", c=64), op=ALU.add)
                else:
                    nc.vector.tensor_copy(vdst, vsrc)

            # ---------- warm-burst: dense MMs to re-arm the PE clock ----------
            wsrc = sb.tile([128, 512], bf16, tag="wsrc")
            nc.vector.memset(wsrc[:].bitcast(mybir.dt.uint16), 0x3C00)

            def warm_burst(nmm=8):
                pw = ps.tile([128, 512], f32, tag="pu", bufs=8)
                for r in range(nmm):
                    nc.tensor.matmul(pw[:], wsrc[:, 0:128], wsrc[:],
                                     start=(r == 0), stop=(r == nmm - 1))

            # ---------- phase 2: chainB, heads interleaved in pairs ----------
            ctxU = [sb.tile([65, S], f16, tag=f"ctxU{h}", name=f"ctxU{h}") for h in range(6)]
            ctxT = [u[0:64, :] for u in ctxU]
            rr_dram = dram.tile([6, S], f16)
            rr2_dram = dram.tile([6, S], f16)

            def chainB_pair(h0):
                pair = (h0, h0 + 1)
                pCc = {}
                for h in pair:
                    pCc[(h, 0)] = ps.tile([65, 512], f32, tag="pu", bufs=8, name=f"pC{h}c0")
                    pCc[(h, 1)] = ps.tile([65, 512], f32, tag="pu", bufs=8, name=f"pC{h}c1")
                seen = {(h, c): 0 for h in pair for c in (0, 1)}
                total = {0: 4, 1: 8}
                for j in range(NB):
                    for h in pair:
                        dt_, hb = h // 2, (h % 2) * 64
                        qh = QTh[dt_][hb:hb + 64, :]
                        kh = KTh[dt_][hb:hb + 64, :]
                        q0 = j * 128
                        first_slab = True
                        while q0 < S:
                            ch = q0 // 512
                            hi = (ch + 1) * 512
                            n = hi - q0
                            pST = ps.tile([128, 512], f32, tag="pu", bufs=8)
                            nc.tensor.matmul(
                                pST[:, 0:n], kh[:, j * 128:(j + 1) * 128], qh[:, q0:q0 + n],
                                start=True, stop=True,
                            )
                            ET = rot.tile([128, 512], f16, tag="ET", bufs=3)
                            if mask_nz:
                                nc.scalar.activation(
                                    ET[:, 0:n], pST[:, 0:n], AF.Exp, scale=SCALE,
                                    bias=mb_colT[:, j:j + 1],
                                )
                            else:
                                nc.scalar.activation(ET[:, 0:n], pST[:, 0:n], AF.Exp, scale=SCALE)
                            if first_slab:
                                nc.gpsimd.affine_select(
                                    out=ET[:, 0:128], in_=ET[:, 0:128],
                                    compare_op=ALU.is_ge, fill=0.0,
                                    base=0, pattern=[[1, 128]], channel_multiplier=-1,
                                )
                            seen[(h, ch)] += 1
                            nc.tensor.matmul(
                                pCc[(h, ch)][:, q0 - ch * 512:q0 - ch * 512 + n],
                                V[j][:, h * 65:(h + 1) * 65],
                                ET[:, 0:n],
                                start=(j == 0),
                                stop=(seen[(h, ch)] == total[ch]),
                            )
                            first_slab = False
                            q0 = hi
                for h in pair:
                    for c in (0, 1):
                        nc.vector.tensor_copy(
                            ctxU[h][:, c * 512:(c + 1) * 512], pCc[(h, c)][:])
                    nc.sync.dma_start(rr_dram[h:h + 1, :], ctxU[h][64:65, :])

            for p in range(3):
                chainB_pair(2 * p)

            # ---------- chainA machinery ----------
            rsums = sb.tile([128, 48], f32, tag="rsums")
            recip = sb.tile([128, 48], f32, tag="recip")

            def chainA_pair(h0):
                for i in range(NB):
                    w = (i + 1) * 128
                    for h in (h0, h0 + 1):
                        dt_, hb = h // 2, (h % 2) * 64
                        qh = QTh[dt_][hb:hb + 64, :]
                        kh = KTh[dt_][hb:hb + 64, :]
                        dlo = i * 128
                        col = h * 8 + i
                        E = rot.tile([128, 1024], f32, tag="E", bufs=3, name=f"E{h}_{i}")
                        nparts = 0
                        for k0 in range(0, w, 512):
                            n = min(512, w - k0)
                            diag_here = dlo >= k0 and dlo < k0 + n
                            pS = ps.tile([128, 512], f32, tag="pu", bufs=8)
                            nc.tensor.matmul(
                                pS[:, 0:n], qh[:, i * 128:(i + 1) * 128], kh[:, k0:k0 + n],
                                start=True, stop=not (diag_here or mask_nz),
                            )
                            if mask_nz:
                                nc.tensor.matmul(
                                    pS[:, 0:n], ones1[:, 0:128], mb_row[:, k0:k0 + n],
                                    start=False, stop=not diag_here,
                                )
                            if diag_here:
                                nc.tensor.matmul(
                                    pS[:, dlo - k0:dlo - k0 + 128], ident_bf[:], cmask[:],
                                    start=False, stop=True,
                                )
                            acc = rsums[:, col:col + 1] if k0 == 0 else rspart[:, h % 2:h % 2 + 1]
                            if k0 > 0:
                                nparts += 1
                            nc.scalar.activation(
                                E[:, k0:k0 + n], pS[:, 0:n], AF.Exp, scale=SCALE,
                                accum_out=acc,
                            )
                        if nparts:
                            nc.vector.tensor_tensor(
                                rsums[:, col:col + 1], rsums[:, col:col + 1],
                                rspart[:, h % 2:h % 2 + 1], op=ALU.add)
                        nc.vector.reciprocal(recip[:, col:col + 1], rsums[:, col:col + 1])
                        P = rot.tile([128, 1024], f32, tag="P", bufs=4, name=f"P{h}_{i}")
                        nc.vector.tensor_scalar(
                            P[:, 0:w], E[:, 0:w], recip[:, col:col + 1], None, op0=ALU.mult,
                        )
                        nc.sync.dma_start(
                            probs_d.ap()[h, i * 128:(i + 1) * 128, 0:w], P[:, 0:w])

            rspart = sb.tile([128, 2], f32, tag="rspart")
            for p in range(3):
                chainA_pair(2 * p)
            warm_burst()

            # ---------- normalize ctx ----------
            rsum6h = sb.tile([6, S], f16, tag="rsum6h")
            rsum6 = sb.tile([6, S], f32, tag="rsum6")
            rrec6 = sb.tile([6, S], f32, tag="rrec6")
            rscr6 = sb.tile([6, S], f32, tag="rscr6")
            rrec6h = sb.tile([6, S], f16, tag="rrec6h")
            nc.sync.dma_start(rsum6h[:], rr_dram[:])
            nc.vector.tensor_copy(rsum6[:], rsum6h[:])
            nc.vector.reciprocal_approx_accurate(rrec6[:], rsum6[:], rscr6[:])
            nc.vector.tensor_copy(rrec6h[:], rrec6[:])
            nc.sync.dma_start(rr2_dram[:], rrec6h[:])
            ra = rr2_dram[:]
            for h in range(6):
                rb = rot.tile([64, S], f16, tag="rb", bufs=3)
                nc.sync.dma_start(
                    rb[:], bass.AP(ra.tensor, ra.offset + h * S, [[0, 64], [1, S]]))
                nc.vector.tensor_tensor(ctxT[h], ctxU[h][0:64, :], rb[:], op=ALU.mult)
                if _dbg:
                    dbgc = rot.tile([64, S], f32, tag="dbgc", bufs=1)
                    nc.vector.tensor_copy(dbgc[:], ctxT[h])
                    nc.sync.dma_start(dbg_ctxT_d.ap()[h], dbgc[:])

            # ---------- projection + ReduceScatter ----------
            partial_dram = dram.tile([S, D], f16)
            ar_dram = dram.tile([SH, D], f16)
            for t in range(NB):
                part = rot.tile([128, D], f16, tag="part", bufs=2)
                for e0, e1 in ((0, 512), (512, 768)):
                    pP = ps.tile([128, 512], f32, tag="pu", bufs=8)
                    for h in range(6):
                        nc.tensor.matmul(
                            pP[:, 0:e1 - e0],
                            ctxT[h][:, t * 128:(t + 1) * 128],
                            woT[h][:, e0:e1],
                            start=(h == 0), stop=(h == 5),
                        )
                    nc.vector.tensor_copy(part[:, e0:e1], pP[:, 0:e1 - e0])
                nc.sync.dma_start(partial_dram[t * 128:(t + 1) * 128, :], part[:])
                if _dbg:
                    nc.sync.dma_start(dbg_partial_d.ap()[t * 128:(t + 1) * 128, :], part[:])

            nc.gpsimd.collective_compute(
                "ReduceScatter", ALU.add,
                replica_groups=[[0, 1], [2, 3], [4, 5], [6, 7]],
                ins=[partial_dram.opt()],
                outs=[ar_dram.opt()],
            )

            # ---------- LayerNorm ----------
            if ln_nz:
                lng = sb.tile([128, D], f32, tag="lng")
                lnb = sb.tile([128, D], f32, tag="lnb")
                a = lnw_d.ap()
                nc.sync.dma_start(lng[:], bass.AP(a.tensor, a.offset, [[0, 128], [1, D]]))
                nc.sync.dma_start(lnb[:], bass.AP(a.tensor, a.offset + D, [[0, 128], [1, D]]))

            for t in range(4):
                ar0 = rot.tile([128, D], f16, tag="ar0", bufs=4)
                nc.sync.dma_start(ar0[:], ar_dram[t * 128:(t + 1) * 128, :])
                emb_sb = rot.tile([128, D], f32, tag="emb_sb", bufs=2)
                nc.sync.dma_start(emb_sb[:], emb_d.ap()[t * 128:(t + 1) * 128, :])
                x_sb = rot.tile([128, D], f32, tag="x_sb", bufs=2)
                sum_x = rot.tile([128, 1], f32, tag="sum_x", bufs=2)
                nc.vector.scalar_tensor_tensor(
                    x_sb[:], ar0[:], 1.0, emb_sb[:], op0=ALU.mult, op1=ALU.add,
                    accum_out=sum_x[:],
                )
                if _dbg:
                    nc.vector.tensor_tensor(ar0[:], x_sb[:], emb_sb[:], op=ALU.subtract)
                    nc.sync.dma_start(dbg_ar_d.ap()[t * 128:(t + 1) * 128, :], ar0[:])
                mu_neg = rot.tile([128, 1], f32, tag="mu_neg", bufs=4)
                nc.vector.tensor_scalar(mu_neg[:], sum_x[:], -1.0 / D, None, op0=ALU.mult)
                sq = rot.tile([128, D], f16, tag="part", bufs=2)
                svar = rot.tile([128, 1], f32, tag="svar", bufs=2)
                nc.scalar.activation(
                    sq[:], x_sb[:], AF.Square, bias=mu_neg[:], accum_out=svar[:],
                )
                nc.vector.tensor_scalar(svar[:], svar[:], 1.0 / D, EPS, op0=ALU.mult, op1=ALU.add)
                srstd = rot.tile([128, 1], f32, tag="srstd", bufs=2)
                nc.scalar.activation(srstd[:], svar[:], AF.Sqrt)
                nc.vector.reciprocal(srstd[:], srstd[:])
                nrm = rot.tile([128, D], f32, tag="nrm", bufs=2)
                nc.vector.tensor_scalar(
                    nrm[:], x_sb[:], mu_neg[:], srstd[:],
                    op0=ALU.add, op1=ALU.mult,
                )
                if ln_nz:
                    nc.vector.tensor_tensor(nrm[:], nrm[:], lng[:], op=ALU.mult)
                    nc.vector.tensor_tensor(nrm[:], nrm[:], lnb[:], op=ALU.add)
                nc.sync.dma_start(normed_d.ap()[t * 128:(t + 1) * 128, :], nrm[:])

    nc.compile()
    return nc


def kernel(embeddings, padding_mask, wq_w, wq_b, wk_w, wk_b, wv_w, wv_b,
           wo_w, wo_b, ln_g, ln_b):
    global _LAST_RES
    embeddings = np.asarray(embeddings, dtype=np.float32)
    padding_mask = np.asarray(padding_mask, dtype=np.float32)
    wq_w = np.asarray(wq_w, dtype=np.float32)
    wk_w = np.asarray(wk_w, dtype=np.float32)
    wv_w = np.asarray(wv_w, dtype=np.float32)
    wo_w = np.asarray(wo_w, dtype=np.float32)
    wq_b = np.asarray(wq_b, dtype=np.float32)
    wk_b = np.asarray(wk_b, dtype=np.float32)
    wv_b = np.asarray(wv_b, dtype=np.float32)
    wo_b = np.asarray(wo_b, dtype=np.float32)
    ln_g = np.asarray(ln_g, dtype=np.float32)
    ln_b = np.asarray(ln_b, dtype=np.float32)

    flags = (
        bool(np.any(wq_b != 0)), bool(np.any(wk_b != 0)), bool(np.any(wv_b != 0)),
        bool(np.any(padding_mask != 1.0)),
        bool(np.any(ln_g != 1.0) or np.any(ln_b != 0.0)),
    )
    if flags not in _CACHE:
        _CACHE[flags] = _build(flags)
    nc = _CACHE[flags]
    qb_nz, kb_nz, vb_nz, mask_nz, ln_nz = flags

    in_maps = []
    for c in range(8):
        b, g = c // 2, c % 2
        hsel = slice(g * HG * HD, (g + 1) * HG * HD)
        m = {
            "xT": np.ascontiguousarray(embeddings[b].T.astype(np.float16)),
            "wqT": np.ascontiguousarray(wq_w[hsel, :].T.astype(np.float16)),
            "wkT": np.ascontiguousarray(wk_w[hsel, :].T.astype(np.float16)),
            "wvT": np.ascontiguousarray(wv_w[hsel, :].T.astype(np.float16)),
            "woT": np.ascontiguousarray(wo_w[:, hsel].T.astype(np.float16)),
            "emb_resid": np.ascontiguousarray(
                embeddings[b, g * SH:(g + 1) * SH, :] + wo_b[None, :]),
        }
        if qb_nz:
            m["qb"] = np.ascontiguousarray(wq_b[hsel].reshape(DG, 1))
        if kb_nz:
            m["kb"] = np.ascontiguousarray(wk_b[hsel].reshape(DG, 1))
        if vb_nz:
            m["vb"] = np.ascontiguousarray(wv_b[hsel].reshape(1, DG))
        if mask_nz:
            mb = (1.0 - padding_mask[b]) * (-10000.0) / SCALE
            m["mb_row"] = np.ascontiguousarray(mb.reshape(1, S))
            m["mb_colT"] = np.ascontiguousarray(
                ((1.0 - padding_mask[b]) * (-10000.0)).reshape(NB, 128).T)
        if ln_nz:
            m["lnw"] = np.ascontiguousarray(np.stack([ln_g, ln_b]))
        in_maps.append(m)

    res = run_bass_kernel_spmd(nc, in_maps, core_ids=list(range(8)))
    _LAST_RES = res

    normed = np.zeros((B, S, D), dtype=np.float32)
    attn_probs = np.zeros((B, H, S, S), dtype=np.float32)
    for c in range(8):
        b, g = c // 2, c % 2
        attn_probs[b, g * HG:(g + 1) * HG] = res.results[c]["probs"]
        normed[b, g * SH:(g + 1) * SH] = res.results[c]["normed"]
    return normed, attn_probs
